# revision 26
# baseline (speedup 1.0000x reference)
"""Trainium2 Bass kernel for nn_DigitCaps (SOM/vq-codebook digit caps layer).

Contract: kernel(**inputs) takes the FULL inputs from setup_inputs()
(inputs [128,12,12,32,8], W [4608,8,32], digit_caps [10,8]) and returns
(output [128,10], digit_caps_new [10,8]) exactly like the reference.

Strategy (J-sharded data-parallel over 8 NeuronCores, fp32 throughout):
  Rather than materializing votes u or doing a per-row segment-sum scatter,
  everything is reformulated as dense matmuls:
    sims[b,(j,m,c)] = sum_i x[b,j,i] * WD[j,i,(m,c)],  WD = W @ dc^T (per m)
    onehot = (sims == rowmax)                      (argmax over c, DVE)
    G[(j,i),(m,c)] = sum_b x[b,(j,i)] * onehot[b,(j,m,c)]   (PE, per-j)
    seg_uT/counts  = sum_{j,i,m-diag} W ∘ G        (PE, psum accumulation)
    u_sumT[d,b]    = sum_{(j,i)} Wm[(j,i),d] x[b,(j,i)],  Wm = sum_m W
  Per-core partials (seg [33,40], usum [8,128]) are summed on host
  (the gather/unshard step), followed by the tiny O(10*8) codebook update
  and the [128,8]x[8,10] output projection.
"""
import os
import sys

import numpy as np

for _p in ("/opt/trn_rl_repo", "/root/.axon_site/_ro/trn_rl_repo"):
    if os.path.isdir(_p) and _p not in sys.path:
        sys.path.insert(0, _p)

import concourse.bass as bass
import concourse.bacc as bacc
import concourse.tile as tile
from concourse import mybir
from concourse.bass_utils import run_bass_kernel_spmd

F32 = mybir.dt.float32

# Problem constants (hardcoded per the harness contract).
H, Wd, INPUT_F, INPUT_D = 12, 12, 32, 8
C, D, M = 10, 8, 4
J = H * Wd * INPUT_F            # 4608
B = 128
N = J * M                       # 18432
NCORES = 8
JL = J // NCORES                # 576 positions per core
TILE_J = 16                     # j's per 128-row (j,i) chunk
NT = JL // TILE_J               # 36 chunks per core
KMC = M * C                     # 40
ROWS = JL * INPUT_D             # 4608 (j,i) rows per core
KAPPA = 1.0 / (B * N)

_CACHE = {}


def _build_nc():
    # Bacc (not plain Bass): its finalize() runs move_matmul_waits_to_
    # ldweights + generate_event_semaphores, which split multi-sem waits
    # into the 1-wait-per-instruction form TRN2 codegen requires.
    nc = bacc.Bacc(None, target_bir_lowering=False)
    # All bulk inputs are host-laid-out partition-major so every DMA is one
    # contiguous run per partition.
    xt_d = nc.declare_dram_parameter("xt", [128, NT * 128], F32, isOutput=False)
    wt_d = nc.declare_dram_parameter("wt", [M * D, ROWS], F32, isOutput=False)
    xe_d = nc.declare_dram_parameter("xe", [B, JL * (INPUT_D + 1)], F32, isOutput=False)
    w4_d = nc.declare_dram_parameter("w4", [128, NT * 4 * 33], F32, isOutput=False)
    w_d = nc.declare_dram_parameter("w", [128, NT * M * D], F32, isOutput=False)
    dcbd_d = nc.declare_dram_parameter("dcbd", [M * D, KMC], F32, isOutput=False)
    seg_d = nc.declare_dram_parameter("seg", [33, KMC], F32, isOutput=True)
    usum_d = nc.declare_dram_parameter("usum", [D, B], F32, isOutput=True)

    with tile.TileContext(nc) as tc:
        with tc.tile_pool(name="big", bufs=1) as big:
            # --- persistent SBUF tiles (live through the main loop) ---
            xt_sb = big.tile([128, NT, 128], F32)       # ((j,i)chunk, t, b)
            xe_sb = big.tile([128, JL * (INPUT_D + 1)], F32)
            w4_sb = big.tile([128, NT * 4, 33], F32)
            # block-diag WD: block jj lives contiguously at slot jj so the
            # diagonal scatter is one contiguous run per partition; the sims
            # matmul reads it back with a strided [jj, mc] AP per tile.
            wdbd_sb = big.tile([128, TILE_J, NT * KMC], F32)
            wm_sb = big.tile([128, NT, D], F32)
            seg_sb = big.tile([33, KMC], F32)
            us_sb = big.tile([D, B], F32)

            # --- phase B: WD production (phase-scoped SBUF) ---
            with tc.tile_pool(name="phase", bufs=1) as phase, \
                 tc.tile_pool(name="psW", bufs=4, space="PSUM") as psW:
                wt_sb = phase.tile([32, NT, 128], F32)
                w_sb = phase.tile([128, NT, M, D], F32)
                dcbd_sb = phase.tile([32, KMC], F32)
                wd_sb = phase.tile([128, NT, KMC], F32)
                # WD-phase inputs first so the WD matmuls start early.
                nc.sync.dma_start(
                    out=wt_sb[:],
                    in_=wt_d.ap().rearrange("md (t b) -> md t b", b=128))
                nc.sync.dma_start(out=dcbd_sb[:], in_=dcbd_d.ap())
                nc.sync.dma_start(
                    out=w_sb[:],
                    in_=w_d.ap().rearrange("p (t m d) -> p t m d", m=M, d=D))
                nc.sync.dma_start(
                    out=xt_sb[:], in_=xt_d.ap().rearrange("p (t b) -> p t b", b=128))
                nc.sync.dma_start(out=xe_sb[:], in_=xe_d.ap())
                nc.sync.dma_start(
                    out=w4_sb[:], in_=w4_d.ap().rearrange("p (s c) -> p s c", c=33))

                # zero the block-diagonal WD buffer (split across engines)
                nc.vector.memset(wdbd_sb[:, 0:TILE_J // 2, :], 0.0)
                nc.gpsimd.memset(wdbd_sb[:, TILE_J // 2:TILE_J, :], 0.0)

                # Wm = sum_m W  -> [128, NT, D]
                nc.vector.tensor_add(wm_sb[:], w_sb[:, :, 0, :], w_sb[:, :, 1, :])
                nc.vector.tensor_add(wm_sb[:], wm_sb[:], w_sb[:, :, 2, :])
                nc.vector.tensor_add(wm_sb[:], wm_sb[:], w_sb[:, :, 3, :])

                for t in range(NT):
                    ps_wd = psW.tile([128, KMC], F32)
                    nc.tensor.matmul(
                        ps_wd[:],
                        lhsT=wt_sb[:, t, :],
                        rhs=dcbd_sb[:],
                        start=True, stop=True)
                    nc.scalar.copy(out=wd_sb[:, t, :], in_=ps_wd[:])

                # --- phase C: scatter WD into block-diag slots ---
                # Contiguous per-partition runs; issued on the ACT HWDGE
                # ring (separate FIFO from the bulk input DMAs on sync).
                flip = 0
                for jj in range(TILE_J):
                    dst = wdbd_sb[jj * 8:(jj + 1) * 8, jj, :].rearrange(
                        "p (t k) -> p t k", k=KMC)
                    src = wd_sb[jj * 8:(jj + 1) * 8, :, :]
                    if jj == 0 or jj == 8:
                        nc.vector.tensor_copy(dst, src)
                    elif jj == 4 or jj == 12:
                        nc.scalar.copy(out=dst, in_=src)
                    else:
                        # t-halved so tiles 0..17 unblock after the first
                        # halves; alternate SWDGE/HWDGE rings.
                        for lo, hi in ((0, NT // 2), (NT // 2, NT)):
                            eng = nc.gpsimd if flip % 2 == 0 else nc.scalar
                            eng.dma_start(out=dst[:, lo:hi, :],
                                          in_=src[:, lo:hi, :])
                            flip += 1



            # --- main loop ---
            with tc.tile_pool(name="psSeg", bufs=1, space="PSUM") as psSegP, \
                 tc.tile_pool(name="psUs", bufs=1, space="PSUM") as psUsP, \
                 tc.tile_pool(name="psS", bufs=2, space="PSUM") as psSP, \
                 tc.tile_pool(name="psG", bufs=1, space="PSUM") as psGP, \
                 tc.tile_pool(name="work", bufs=3) as work, \
                 tc.tile_pool(name="gbp", bufs=4) as gbp:
                ps_seg = psSegP.tile([33, KMC], F32)
                ps_us = psUsP.tile([D, B], F32)
                # ping-pong G psums; memset once so the never-written rows
                # (9..31 of each 32-strip) are defined — step-2 multiplies
                # them by w4's zero rows, so their value is irrelevant.
                ps_g0 = psGP.tile([128, KMC], F32, tag="psg0")
                ps_g1 = psGP.tile([128, KMC], F32, tag="psg1")
                nc.vector.memset(ps_g0[:], 0.0)
                nc.vector.memset(ps_g1[:], 0.0)
                ps_gs = (ps_g0, ps_g1)

                # u_sum accumulation: independent of sims/onehot — issued
                # first so PE has work while the WD diagonal scatter runs.
                for t in range(NT):
                    nc.tensor.matmul(
                        ps_us[:], lhsT=wm_sb[:, t, :], rhs=xt_sb[:, t, :],
                        start=(t == 0), stop=(t == NT - 1),
                        skip_group_check=True)

                for t in range(NT):
                    ps_a = psSP.tile([128, 320], F32, tag="psA")
                    ps_b = psSP.tile([128, 320], F32, tag="psB")
                    nc.tensor.matmul(
                        ps_a[:], lhsT=xt_sb[:, t, :],
                        rhs=wdbd_sb[:, 0:TILE_J // 2, t * KMC:(t + 1) * KMC],
                        start=True, stop=True)
                    nc.tensor.matmul(
                        ps_b[:], lhsT=xt_sb[:, t, :],
                        rhs=wdbd_sb[:, TILE_J // 2:TILE_J,
                                    t * KMC:(t + 1) * KMC],
                        start=True, stop=True)

                    oh = work.tile([128, TILE_J * KMC], F32, tag="oh")
                    for h, ps_h in ((0, ps_a), (1, ps_b)):
                        ps3 = ps_h[:].rearrange("p (r c) -> p r c", c=C)
                        rm = work.tile([128, 32], F32, tag="rm")
                        nc.vector.tensor_reduce(
                            out=rm[:], in_=ps3,
                            axis=mybir.AxisListType.X, op=mybir.AluOpType.max)
                        rm_ap = rm[:]
                        rm_b = bass.AP(
                            tensor=rm_ap.tensor, offset=rm_ap.offset,
                            ap=[rm_ap.ap[0], rm_ap.ap[1], [0, C]])
                        oh3 = oh[:, h * 320:(h + 1) * 320].rearrange(
                            "p (r c) -> p r c", c=C)
                        nc.vector.tensor_tensor(
                            oh3, ps3, rm_b, mybir.AluOpType.is_equal)

                    # G matmuls + step-2 accumulation
                    for g in range(4):
                        ps_g = ps_gs[(t * 4 + g) % 2]
                        for jjl in range(4):
                            jj = 4 * g + jjl
                            jloc = t * TILE_J + jj
                            nc.tensor.matmul(
                                ps_g[32 * jjl:32 * jjl + 9, :],
                                lhsT=xe_sb[:, jloc * 9:(jloc + 1) * 9],
                                rhs=oh[:, jj * KMC:(jj + 1) * KMC],
                                start=True, stop=True,
                                tile_position=(0, 32 * jjl))
                        gb = gbp.tile([128, KMC], F32, tag="gb")
                        nc.scalar.copy(out=gb[:], in_=ps_g[:])
                        s = t * 4 + g
                        nc.tensor.matmul(
                            ps_seg[:], lhsT=w4_sb[:, s, :], rhs=gb[:],
                            start=(s == 0), stop=(s == NT * 4 - 1),
                            skip_group_check=True)

                nc.vector.tensor_copy(seg_sb[:], ps_seg[:])
                nc.vector.tensor_copy(us_sb[:], ps_us[:])
            nc.sync.dma_start(out=seg_d.ap(), in_=seg_sb[:])
            nc.sync.dma_start(out=usum_d.ap(), in_=us_sb[:])
    if not nc.is_finalized():
        nc.finalize()
    return nc


def _host_prep(inputs, W, digit_caps):
    x = np.ascontiguousarray(np.asarray(inputs, np.float32)).reshape(B, J, INPUT_D)
    W = np.asarray(W, np.float32)
    dc = np.ascontiguousarray(np.asarray(digit_caps, np.float32))
    dcbd = np.zeros((M * D, KMC), np.float32)
    for m in range(M):
        dcbd[m * D:(m + 1) * D, m * C:(m + 1) * C] = dc.T
    in_maps = []
    for k in range(NCORES):
        xs = x[:, k * JL:(k + 1) * JL, :]
        xc = xs.reshape(B, ROWS)
        xt = np.ascontiguousarray(xc.T)
        w = np.ascontiguousarray(W[k * JL:(k + 1) * JL].reshape(ROWS, D * M))
        wt = np.ascontiguousarray(w.T)
        xe = np.ones((B, JL, INPUT_D + 1), np.float32)
        xe[:, :, :INPUT_D] = xs
        xe = np.ascontiguousarray(xe.reshape(B, JL * (INPUT_D + 1)))
        w4 = np.zeros((NT * 4, 128, 33), np.float32)
        wr = W[k * JL:(k + 1) * JL].reshape(JL, INPUT_D, D * M)
        for jjl in range(4):
            # rows 32*jjl + i  <- W[j = t*16 + 4g + jjl]
            w4[:, 32 * jjl:32 * jjl + INPUT_D, :32] = (
                wr.reshape(NT, 4, 4, INPUT_D, D * M)[:, :, jjl]
                .reshape(NT * 4, INPUT_D, D * M))
            w4[:, 32 * jjl + INPUT_D, 32] = 1.0
        in_maps.append({
            "xt": np.ascontiguousarray(
                xt.reshape(NT, 128, 128).transpose(1, 0, 2).reshape(128, -1)),
            "wt": wt, "xe": xe,
            "w4": np.ascontiguousarray(
                w4.transpose(1, 0, 2).reshape(128, -1)),
            "w": np.ascontiguousarray(
                w.reshape(NT, 128, M * D).transpose(1, 0, 2).reshape(128, -1)),
            "dcbd": dcbd,
        })
    return in_maps, dc


def _epilogue(seg, usumT, dc):
    seg_uT = np.zeros((D, C), np.float32)
    cnt = np.zeros((C,), np.float32)
    for m in range(M):
        seg_uT += seg[m * D:(m + 1) * D, m * C:(m + 1) * C]
        cnt += seg[32, m * C:(m + 1) * C]
    dcT = dc.T
    dc_newT = dcT + (seg_uT - dcT * cnt[None, :]) * np.float32(KAPPA)
    output = (usumT.T @ dc_newT) / np.float32(N)
    return output.astype(np.float32), np.ascontiguousarray(dc_newT.T)


def kernel(inputs, W, digit_caps, _trace=False, _trace_kwargs=None):
    in_maps, dc = _host_prep(inputs, W, digit_caps)
    if "nc" not in _CACHE:
        _CACHE["nc"] = _build_nc()
    nc = _CACHE["nc"]
    kw = {}
    if _trace:
        kw = dict(trace=True, **(_trace_kwargs or {}))
    res = run_bass_kernel_spmd(nc, in_maps, list(range(NCORES)), **kw)
    outs = res.results
    seg = np.sum([o["seg"] for o in outs], axis=0, dtype=np.float64).astype(np.float32)
    usumT = np.sum([o["usum"] for o in outs], axis=0, dtype=np.float64).astype(np.float32)
    out, dc_new = _epilogue(seg, usumT, dc)
    if _trace:
        kernel._last_result = res
    return out, dc_new
